# revision 22
# baseline (speedup 1.0000x reference)
"""MoE gating kernel for Trainium2 (Bass/Tile), data-parallel over 8 NeuronCores.

Computes: logits = x @ W_g.T ; top-2 values; softmax over the 2 values.
  p1 = sigmoid(v1 - v2), p2 = sigmoid(v2 - v1)  (v1 >= v2 the top-2 logits)

Sharding: tokens split 8 ways (2048 tokens/core), W_g replicated.

Measured ~60.5us (was 73.2us for the previous version of this kernel).

Design notes — the stream is the roofline and the DGE details rule it:
  - The 16 SDMA engines process one 8KB fp32 row per ~306ns each (~27
    GB/s/engine, fabric-capped); 2048 rows = ~39.2us aggregate on ANY DGE
    path, and engines round-robin the active queue-rings fairly at packet
    granularity — so total stream time is conserved; only the start time
    and the per-tile ARRIVAL ORDER are controllable.
  - Two constraints that serialized earlier attempts by 20-30us each:
    (a) every engine FIFO executes its program in fixed order, so one
    instruction gated on a late tile arrival blocks every ready
    instruction queued behind it on that engine; (b) the Tile scheduler
    has 8 DMAHW completion-sem lanes for HWDGE DMAs — a 9th in-flight
    HWDGE DMA carries a lane-reuse wait that stalls the issuing
    sequencer's whole FIFO.
  - Hence: the SWDGE pool ring carries 12 tiles in process order
    (in-order ~2.5us/tile cadence once solo; its first descriptor comes
    ~9us in, after the Q7 boot); HWDGE (sync+scalar rings, dispatching
    from ~6.8us) carries 4 tiles that bridge the boot gap. Process order
    is the expected arrival order [0,1,2,14,3,15,4..13]; the host
    unpermutes the output (free). The last two pool tiles are split into
    d-halves (two DMAs, separate buffers) so their transposes/mains
    start ~1.2us before the full tile lands.
  - The 4 HWDGE tiles arrive fp32 and are cast to bf16 whole-tile (2 on
    DVE / 2 on ACT) at FIFO positions matching their arrival, so the
    casts never block drains queued behind them.
  - Compute is fully per-tile (no group barrier): 16 bf16 transposes vs
    identity (REGULAR matmuls — HAM-visible, so the PE clock gate stays
    at 2.4GHz), PSUM drains alternating DVE/ACT, then 16 x-STATIONARY
    mains (the 128-col xT slice is the FWL-loaded stationary, the tiny
    [128,64] wgT is the moving operand) accumulating token-major logits
    [128t, 64e] in PSUM — max8 then reads PSUM directly. This removes
    the logitsT->token-major transpose-back, the lt copies, and makes
    the serial tail after the last byte one tile's chain (~2.5us).
  - N=512 warm-up matmuls alternating two PSUM banks flip the HAM clock
    gate after PE boot; a few keepers bridge early arrival gaps. The
    sigmoid ACT table is preloaded early on a memset scratch so the tail
    doesn't pay the 1.28us ACT_TABLE_LOAD. One batched sigmoid + one
    contiguous partition-major store, dispatched from the scalar ring so
    no cross-engine hop follows the sigmoid.
bf16 adds ~4e-3 relative error on the output probabilities (gate 2e-2).
"""

import sys

sys.path.insert(0, "/opt/trn_rl_repo")

from contextlib import ExitStack

import numpy as np
import ml_dtypes

import concourse.bass as bass
import concourse.bacc as bacc
import concourse.mybir as mybir
from concourse.tile import TileContext
from concourse.bass_utils import run_bass_kernel_spmd

TOKENS = 16384
DIM = 2048
E = 64  # num experts
NCORES = 8
TPC = TOKENS // NCORES  # tokens per core
P = 128
KT = DIM // P  # 16 contraction tiles
NB = TPC // P  # 16 token blocks (tiles) per core
HD = DIM // 2  # half-tile split for the last pool tiles

F32 = mybir.dt.float32
BF16 = mybir.dt.bfloat16
N_WARM = 16

SYNC_TILES = (0, 14)  # HWDGE sync ring, ring order
ACT_TILES = (1, 15)  # HWDGE scalar ring, ring order
S_TILES = tuple(range(2, 14))  # SWDGE pool ring, ring order
SPLIT_TILES = frozenset({12, 13})  # pool tiles DMA'd as two d-halves
CAST_DVE = frozenset({0, 14})  # whole-tile cast engine per HWDGE tile
# process order == expected arrival order; host unpermutes
PROC_ORDER = (0, 1, 2, 14, 3, 15, 4, 5, 6, 7, 8, 9, 10, 11, 12, 13)


def _emit(tc, ctx, x_ap, wgt_ap, idb_ap, out_ap):
    nc = tc.nc

    singles = ctx.enter_context(tc.tile_pool(name="singles", bufs=1))
    xtpool = ctx.enter_context(tc.tile_pool(name="xtpool", bufs=4))
    spool = ctx.enter_context(tc.tile_pool(name="spool", bufs=4))
    psum_t = ctx.enter_context(tc.tile_pool(name="psum_t", bufs=3, space="PSUM"))
    psum_l = ctx.enter_context(tc.tile_pool(name="psum_l", bufs=2, space="PSUM"))
    psum_f = ctx.enter_context(tc.tile_pool(name="psum_f", bufs=1, space="PSUM"))
    psum_w = ctx.enter_context(tc.tile_pool(name="psum_w", bufs=1, space="PSUM"))

    warm = singles.tile([P, P], BF16)
    warm_rhs = singles.tile([P, 4 * P], BF16)
    sig_scratch = singles.tile([1, 2], F32)
    nc.vector.memset(warm[:], 0.0)
    nc.vector.memset(warm_rhs[:], 0.0)
    nc.vector.memset(sig_scratch[:], 0.0)

    warm_flip = [False]

    def warm_mm():
        # alternate PSUM banks: back-to-back matmuls into ONE bank
        # serialize on the write-after-write; alternating sustains the
        # ~80% duty HAM needs to flip
        warm_flip[0] = not warm_flip[0]
        if warm_flip[0]:
            pw = psum_w.tile([P, 4 * P], F32, tag="warm_ps")
        else:
            pw = psum_f.tile([P, 4 * P], F32, tag="fin_ps")
        nc.tensor.matmul(pw[:], warm[:], warm_rhs[:])

    for _ in range(N_WARM):
        warm_mm()

    def keeper(n=1):
        for _ in range(n):
            warm_mm()

    xf32 = {}
    xb = {}
    for t in range(NB):
        if t in SPLIT_TILES:
            xb[t] = [
                singles.tile([P, HD], BF16, tag=f"xb{t}a", name=f"xb{t}a"),
                singles.tile([P, HD], BF16, tag=f"xb{t}b", name=f"xb{t}b"),
            ]
        else:
            xb[t] = [singles.tile([P, DIM], BF16, tag=f"xb{t}", name=f"xb{t}")]
    for t in SYNC_TILES + ACT_TILES:
        xf32[t] = singles.tile([P, DIM], F32, tag=f"xf{t}", name=f"xf{t}")

    ident = singles.tile([P, P], BF16)
    wgT = singles.tile([P, KT, E], BF16)

    def xb_chunk(t, k):
        # the [128, 128] k-th d-chunk of tile t's bf16 buffer(s)
        if t in SPLIT_TILES:
            half = xb[t][k // (KT // 2)]
            kk = k % (KT // 2)
            return half[:, kk * P : (kk + 1) * P]
        return xb[t][0][:, k * P : (k + 1) * P]

    # pool ring: identity first (transposes need it), then x tiles in
    # process order with wgT slotted before the first mains need it
    nc.gpsimd.dma_start(out=ident[:], in_=idb_ap)
    nc.gpsimd.dma_start(out=wgT[:], in_=wgt_ap)
    for t in S_TILES:
        if t in SPLIT_TILES:
            nc.gpsimd.dma_start(out=xb[t][0][:], in_=x_ap[t * P : (t + 1) * P, 0:HD])
            nc.gpsimd.dma_start(out=xb[t][1][:], in_=x_ap[t * P : (t + 1) * P, HD:DIM])
        else:
            nc.gpsimd.dma_start(out=xb[t][0][:], in_=x_ap[t * P : (t + 1) * P, :])
    for t in SYNC_TILES:
        nc.sync.dma_start(out=xf32[t][:], in_=x_ap[t * P : (t + 1) * P, :])
    for t in ACT_TILES:
        nc.scalar.dma_start(out=xf32[t][:], in_=x_ap[t * P : (t + 1) * P, :])

    # per-process-position (v1-v2, v2-v1) accumulate here
    dd_all = singles.tile([P, NB, 2], F32)
    sig_preloaded = [False]

    def cast_tile(t):
        if t in CAST_DVE:
            nc.vector.tensor_copy(xb[t][0][:], xf32[t][:])
        else:
            nc.scalar.copy(xb[t][0][:], xf32[t][:])
        if not sig_preloaded[0]:
            sig_preloaded[0] = True
            nc.scalar.activation(
                sig_scratch[:], sig_scratch[:], mybir.ActivationFunctionType.Sigmoid
            )

    for pos in range(NB):
        t = PROC_ORDER[pos]
        if t in xf32:
            cast_tile(t)
        if 2 <= pos < 8 and pos % 2 == 0:
            keeper(1)

        # 16 regular bf16 transposes vs identity -> xt_t [128d-slices, t],
        # then x-stationary mains: logits land token-major [128t, 64e] in
        # PSUM. For d-split tiles the first half's mains are emitted
        # before the second half's transposes, so the PE isn't FIFO-
        # blocked on the late half while ready mains wait.
        xt_t = xtpool.tile([P, KT * P], BF16)
        fp = psum_l.tile([P, E], F32)

        def transpose_quads(quads):
            for q in quads:
                pt = psum_t.tile([P, 4 * P], F32)
                for j in range(4):
                    k = 4 * q + j
                    nc.tensor.matmul(
                        pt[:, j * P : (j + 1) * P],
                        xb_chunk(t, k),
                        ident[:],
                    )
                dst = xt_t[:, 4 * q * P : (4 * q + 4) * P]
                if q % 2 == 0:
                    nc.vector.tensor_copy(dst, pt[:])
                else:
                    nc.scalar.copy(dst, pt[:])

        def mains(ks):
            for k in ks:
                nc.tensor.matmul(
                    fp[:],
                    xt_t[:, k * P : (k + 1) * P],
                    wgT[:, k, :],
                    start=(k == 0),
                    stop=(k == KT - 1),
                )

        if t in SPLIT_TILES:
            transpose_quads([0, 1])
            mains(range(KT // 2))
            transpose_quads([2, 3])
            mains(range(KT // 2, KT))
        else:
            transpose_quads(range(KT // 4))
            mains(range(KT))
        max8 = spool.tile([P, 8], F32)
        nc.vector.max(out=max8[:], in_=fp[:])
        nc.vector.tensor_sub(dd_all[:, pos, 0:1], max8[:, 0:1], max8[:, 1:2])
        nc.vector.tensor_sub(dd_all[:, pos, 1:2], max8[:, 1:2], max8[:, 0:1])

    # single sigmoid + one contiguous partition-major store
    ot = singles.tile([P, NB, 2], F32)
    nc.scalar.activation(ot[:], dd_all[:], mybir.ActivationFunctionType.Sigmoid)
    nc.scalar.dma_start(out=out_ap, in_=ot[:])


_NC_CACHE = {}


def _build():
    key = "nc"
    if key in _NC_CACHE:
        return _NC_CACHE[key]
    nc = bacc.Bacc(trn_type="TRN2")
    x = nc.dram_tensor("x", [TPC, DIM], F32, kind="ExternalInput")
    wgt = nc.dram_tensor("wgt", [P, KT * E], BF16, kind="ExternalInput")
    idb = nc.dram_tensor("idb", [P, P], BF16, kind="ExternalInput")
    out = nc.dram_tensor("out", [P, NB * 2], F32, kind="ExternalOutput")
    with TileContext(nc) as tc, ExitStack() as ctx:
        _emit(tc, ctx, x.ap(), wgt.ap(), idb.ap(), out.ap())
    if not nc.is_finalized():
        nc.finalize()
    _NC_CACHE[key] = nc
    return nc


def _run(x, W_g, trace=False):
    nc = _build()
    x = np.ascontiguousarray(np.asarray(x, dtype=np.float32))
    W_g = np.asarray(W_g, dtype=np.float32)
    # host-side weight layout prep: wgt[p, k*E + e] = W_g[e, k*128 + p]
    wgt = np.ascontiguousarray(
        W_g.reshape(E, KT, P).transpose(2, 1, 0).reshape(P, KT * E)
    ).astype(ml_dtypes.bfloat16)
    idb = np.eye(P, dtype=np.float32).astype(ml_dtypes.bfloat16)
    in_maps = [
        {
            "x": np.ascontiguousarray(x[c * TPC : (c + 1) * TPC]),
            "wgt": wgt,
            "idb": idb,
        }
        for c in range(NCORES)
    ]
    res = run_bass_kernel_spmd(nc, in_maps, core_ids=list(range(NCORES)), trace=trace)
    # device output is partition-major [128, 16, 2] in PROCESS order;
    # de-interleave + unpermute: out[PROC_ORDER[b]*128 + p] = res[p, b]
    inv = np.argsort(np.array(PROC_ORDER))
    outs = []
    for r in res.results:
        o = r["out"].reshape(P, NB, 2)[:, inv, :].transpose(1, 0, 2).reshape(TPC, 2)
        outs.append(o)
    out = np.ascontiguousarray(np.concatenate(outs, axis=0))
    return out, res


def kernel(x, W_g):
    out, _ = _run(x, W_g, trace=False)
    return out


def kernel_profiled(x, W_g, **_kw):
    out, res = _run(x, W_g, trace=True)
    return out, res
